# revision 5
# baseline (speedup 1.0000x reference)
"""Adaptive avg pool 2D (16,768,64,48) -> (16,768,7,7) on 8 TRN2 NeuronCores.

Data-parallel over B*C rows: 1536 rows/core, 12 tiles of 128 rows. The DMA
device is the bottleneck (52.4us of loads at the modeled 360B/ns), so the
kernel is built around a gap-free load stream plus a minimal post-load tail:

- The first load's dma_start is hoisted into the SP preamble (before the
  entry barrier) so the DMA device starts at ~1.55us instead of ~2.3us.
- Loads stay alone on the 8 HWDGE issue lanes (a HWDGE DMA's issue waits
  its lane predecessor's completion, so a store in the rotation would stall
  later loads), and every store is emitted late enough that its DMA-FIFO
  request trails the last load request.
- Steady tiles 0..9: H-pool (all windows 10 rows, stride 9) split across
  engines - DVE windowed reduce for w in [0,39), Pool as a 9-add chain for
  w in [39,48); W-pool as two DVE reduces (q in {0,6}: size-7 windows,
  q in 1..5: size-8); scales 1/70, 1/80 on Act into a [128, 588] staging
  tile. Completion trails each tile's load by ~4.3us.
- Tile 10 loads as o0-5 + o6 h-chunks, tile 11 as o0-2 / o3-5 / o6, so the
  compute after the final 683ns o6 load is just one fused (h,w) reduce
  pair plus two tiny DVE scales.
- Stores: tiles 0-7 and 8-10 as two batched Act DMAs (explicit 3-dim DRAM
  APs - row = 128*tile + partition), tile 11 as one SP DMA at the very end.
- A post-pass drops the unused SWDGE-ring preamble memsets (no gpsimd
  DMAs here), another hoists the first load, and _legalize_multiwait
  splits multi-wait sync_infos for this walrus (max 1 wait/instruction),
  ordering the final store's lane sem last so the end drain does not
  trail it with already-satisfied waits.

Cost-model timeline: 60444 ns/core (baseline 62571; HBM floor ~52.6us).
"""

import sys

_TRN_REPO = "/opt/trn_rl_repo"
if _TRN_REPO not in sys.path:
    sys.path.insert(0, _TRN_REPO)

import numpy as np

import concourse.bass as bass
import concourse.mybir as mybir
from concourse.tile import TileContext

B, C, H, W = 16, 768, 64, 48
HO, WO = 7, 7
NCORES = 8
ROWS = B * C // NCORES  # 1536
P = 128
NTILES = ROWS // P  # 12
WD = 39  # steady: DVE H cols [0, WD), Pool cols [WD, 48)
f32 = mybir.dt.float32
X = mybir.AxisListType.X
XY = mybir.AxisListType.XY

_nc_cache = None


def _legalize_multiwait(nc: bass.Bass) -> None:
    """Walrus accepts at most one sync wait per instruction (two for
    EventSemaphore). Hoist extra waits into single-wait EventSemaphore
    carriers placed directly before the offending instruction."""
    n = 0
    final_lane = None
    for b in nc.m.functions[0].blocks:
        for inst in b.instructions:
            if type(inst).__name__ == "InstDMACopy" and inst.sync_info:
                for w in inst.sync_info.on_wait:
                    if (w.ant_name or "").startswith("DMAHW"):
                        final_lane = w.ant_name
    for b in nc.m.functions[0].blocks:
        insts = b.instructions
        i = 0
        while i < len(insts):
            inst = insts[i]
            si = inst.sync_info
            if si is not None and len(si.on_wait) > 1:
                waits = sorted(
                    si.on_wait,
                    key=lambda w: (
                        2
                        if w.ant_name == final_lane
                        else 1
                        if (w.ant_name or "").startswith("DMA")
                        else 0
                    ),
                )
                carriers = []
                rest = waits[:-1]
                for j in range(0, len(rest), 2):
                    n += 1
                    ev = mybir.InstEventSemaphore(
                        name=f"I-waitfix-{n}", ins=[], outs=[]
                    )
                    ev.engine = inst.engine
                    ev.sync_info = mybir.SyncInfo(
                        on_wait=rest[j : j + 2], on_update=[]
                    )
                    nc.register_instruction(ev)
                    carriers.append(ev)
                inst.sync_info = mybir.SyncInfo(
                    on_wait=[waits[-1]], on_update=list(si.on_update)
                )
                insts[i:i] = carriers
                i += len(carriers)
            i += 1



def _strip_pool_ring_memsets(nc: bass.Bass) -> None:
    """The framework preamble memsets the SWDGE descriptor rings on Pool.
    This kernel issues no SWDGE DMAs (loads ride SP, stores Act/SP via
    HWDGE), so the ring init only delays the entry barrier; drop it."""
    for b in nc.m.functions[0].blocks:
        keep = [
            i
            for i in b.instructions
            if not (
                type(i).__name__ == "InstMemset"
                and str(i.engine).endswith("Pool")
                and i.name in ("I-29", "I-30", "I-31", "I-32")
            )
        ]
        if len(keep) != len(b.instructions):
            b.instructions[:] = keep



def _hoist_first_load(nc: bass.Bass) -> None:
    """Move the first load's dma_start into the preamble block, right after
    SP's queue-setup RegisterMoves and before the entry barrier. The load
    has no waits and its consumers are semaphore-gated, so it can issue
    while the other engines are still running their preambles (~0.5us
    earlier DMA start)."""
    blocks = nc.m.functions[0].blocks
    if len(blocks) < 2:
        return
    b0, b1 = blocks[0], blocks[1]
    first = None
    for i in b1.instructions:
        if type(i).__name__ == "InstDMACopy" and str(i.engine).endswith("SP"):
            if i.sync_info and i.sync_info.on_wait:
                return  # unexpected: keep conservative
            first = i
            break
    if first is None:
        return
    b1.instructions.remove(first)
    idx = next(
        (k for k, i in enumerate(b0.instructions) if type(i).__name__ == "InstDrain"),
        None,
    )
    if idx is None:
        b1.instructions.insert(0, first)
        return
    b0.instructions.insert(idx, first)


def _build() -> bass.Bass:
    nc = bass.Bass()
    x = nc.dram_tensor("x", [ROWS, H * W], f32, kind="ExternalInput")
    out = nc.dram_tensor("out", [ROWS, HO * WO], f32, kind="ExternalOutput")
    ACopy = mybir.ActivationFunctionType.Copy
    with TileContext(nc) as tc:
        with (
            tc.tile_pool(name="xp", bufs=NTILES - 2) as xp,
            tc.tile_pool(name="cp", bufs=1) as cpool,
            tc.tile_pool(name="tp", bufs=3) as tp,
            tc.tile_pool(name="wp", bufs=3) as wp,
            tc.tile_pool(name="op", bufs=1) as op,
        ):
            nfull = NTILES - 2
            os_ = op.tile([P, NTILES * HO * WO], f32)  # [128, 588] staging
            po = list(os_.ap[0])

            # --- loads: 10 full tiles; tile 10 as o0-5 + o6 h-chunks;
            #     tile 11 as o0-2 / o3-5 / o6 h-chunks ---------------------
            xt = []
            for i in range(nfull):
                t = xp.tile([P, H, W], f32)
                nc.sync.dma_start(
                    out=t,
                    in_=x[i * P : (i + 1) * P, :].rearrange(
                        "p (h w) -> p h w", w=W
                    ),
                )
                xt.append(t)
            rows10 = x[nfull * P : (nfull + 1) * P, :].rearrange(
                "p (h w) -> p h w", w=W
            )
            xa10 = cpool.tile([P, 55, W], f32, tag="xa10")
            nc.sync.dma_start(out=xa10, in_=rows10[:, 0:55, :])
            xc10 = cpool.tile([P, 10, W], f32, tag="xc10")
            nc.sync.dma_start(out=xc10, in_=rows10[:, 54:64, :])
            rows11 = x[(nfull + 1) * P :, :].rearrange("p (h w) -> p h w", w=W)
            xa = cpool.tile([P, 28, W], f32, tag="xa")
            nc.sync.dma_start(out=xa, in_=rows11[:, 0:28, :])
            xb = cpool.tile([P, 28, W], f32, tag="xb")
            nc.sync.dma_start(out=xb, in_=rows11[:, 27:55, :])
            xc = cpool.tile([P, 10, W], f32, tag="xc")
            nc.sync.dma_start(out=xc, in_=rows11[:, 54:64, :])

            # --- helpers -------------------------------------------------
            def h_pool(xtile, tH):
                pt = list(xtile.ap[0])
                ph = list(tH.ap[0])
                nc.vector.reduce_sum(
                    out=bass.AP(
                        tensor=tH.tensor,
                        offset=tH.offset,
                        ap=[ph, [W, HO], [1, WD]],
                    ),
                    in_=bass.AP(
                        tensor=xtile.tensor,
                        offset=xtile.offset,
                        ap=[pt, [9 * W, HO], [1, WD], [W, 10]],
                    ),
                    axis=X,
                )
                tsl = bass.AP(
                    tensor=tH.tensor,
                    offset=tH.offset + WD,
                    ap=[ph, [W, HO], [1, W - WD]],
                )

                def xsl(k):
                    return bass.AP(
                        tensor=xtile.tensor,
                        offset=xtile.offset + k * W + WD,
                        ap=[pt, [9 * W, HO], [1, W - WD]],
                    )

                nc.gpsimd.tensor_add(tsl, xsl(0), xsl(1))
                for k in range(2, 10):
                    nc.gpsimd.tensor_add(tsl, tsl, xsl(k))

            def w_pool(tH, wout):
                ph = list(tH.ap[0])
                pw = list(wout.ap[0])
                nc.vector.reduce_sum(
                    out=bass.AP(
                        tensor=wout.tensor,
                        offset=wout.offset,
                        ap=[pw, [WO, HO], [6, 2]],
                    ),
                    in_=bass.AP(
                        tensor=tH.tensor,
                        offset=tH.offset,
                        ap=[ph, [W, HO], [41, 2], [1, 7]],
                    ),
                    axis=X,
                )
                nc.vector.reduce_sum(
                    out=bass.AP(
                        tensor=wout.tensor,
                        offset=wout.offset + 1,
                        ap=[pw, [WO, HO], [1, 5]],
                    ),
                    in_=bass.AP(
                        tensor=tH.tensor,
                        offset=tH.offset + 6,
                        ap=[ph, [W, HO], [7, 5], [1, 8]],
                    ),
                    axis=X,
                )

            def act_scales(wout, obase, o0, no):
                pw = list(wout.ap[0])
                nc.scalar.activation(
                    out=bass.AP(
                        tensor=os_.tensor,
                        offset=os_.offset + obase + 7 * o0,
                        ap=[po, [WO, no], [6, 2]],
                    ),
                    in_=bass.AP(
                        tensor=wout.tensor,
                        offset=wout.offset + 7 * o0,
                        ap=[pw, [WO, no], [6, 2]],
                    ),
                    func=ACopy,
                    scale=1.0 / 70.0,
                )
                nc.scalar.activation(
                    out=bass.AP(
                        tensor=os_.tensor,
                        offset=os_.offset + obase + 7 * o0 + 1,
                        ap=[po, [WO, no], [1, 5]],
                    ),
                    in_=bass.AP(
                        tensor=wout.tensor,
                        offset=wout.offset + 7 * o0 + 1,
                        ap=[pw, [WO, no], [1, 5]],
                    ),
                    func=ACopy,
                    scale=1.0 / 80.0,
                )

            def chunk_hw(xtile, tH, wout, o0, no, h0, do_w=True):
                """H-pool rows [o0,o0+no) split DVE/Pool, then W-pool."""
                pt = list(xtile.ap[0])
                ph = list(tH.ap[0])
                pw = list(wout.ap[0])
                off = (9 * o0 - h0) * W
                nc.vector.reduce_sum(
                    out=bass.AP(
                        tensor=tH.tensor,
                        offset=tH.offset + o0 * W,
                        ap=[ph, [W, no], [1, WD]],
                    ),
                    in_=bass.AP(
                        tensor=xtile.tensor,
                        offset=xtile.offset + off,
                        ap=[pt, [9 * W, no], [1, WD], [W, 10]],
                    ),
                    axis=X,
                )
                tsl = bass.AP(
                    tensor=tH.tensor,
                    offset=tH.offset + o0 * W + WD,
                    ap=[ph, [W, no], [1, W - WD]],
                )

                def xsl(k):
                    return bass.AP(
                        tensor=xtile.tensor,
                        offset=xtile.offset + off + k * W + WD,
                        ap=[pt, [9 * W, no], [1, W - WD]],
                    )

                nc.gpsimd.tensor_add(tsl, xsl(0), xsl(1))
                for k in range(2, 10):
                    nc.gpsimd.tensor_add(tsl, tsl, xsl(k))
                if not do_w:
                    return
                nc.vector.reduce_sum(
                    out=bass.AP(
                        tensor=wout.tensor,
                        offset=wout.offset + 7 * o0,
                        ap=[pw, [WO, no], [6, 2]],
                    ),
                    in_=bass.AP(
                        tensor=tH.tensor,
                        offset=tH.offset + o0 * W,
                        ap=[ph, [W, no], [41, 2], [1, 7]],
                    ),
                    axis=X,
                )
                nc.vector.reduce_sum(
                    out=bass.AP(
                        tensor=wout.tensor,
                        offset=wout.offset + 7 * o0 + 1,
                        ap=[pw, [WO, no], [1, 5]],
                    ),
                    in_=bass.AP(
                        tensor=tH.tensor,
                        offset=tH.offset + o0 * W + 6,
                        ap=[ph, [W, no], [7, 5], [1, 8]],
                    ),
                    axis=X,
                )

            def fused_chunk(xtile, wout, o0, no, h0):
                """wout[:, 7o+q] for o in [o0,o0+no) by 2D (h,w) reduces."""
                pc = list(xtile.ap[0])
                pw = list(wout.ap[0])
                off = (9 * o0 - h0) * W
                nc.vector.reduce_sum(
                    out=bass.AP(
                        tensor=wout.tensor,
                        offset=wout.offset + 7 * o0,
                        ap=[pw, [WO, no], [6, 2]],
                    ),
                    in_=bass.AP(
                        tensor=xtile.tensor,
                        offset=xtile.offset + off,
                        ap=[pc, [9 * W, no], [41, 2], [W, 10], [1, 7]],
                    ),
                    axis=XY,
                )
                nc.vector.reduce_sum(
                    out=bass.AP(
                        tensor=wout.tensor,
                        offset=wout.offset + 7 * o0 + 1,
                        ap=[pw, [WO, no], [1, 5]],
                    ),
                    in_=bass.AP(
                        tensor=xtile.tensor,
                        offset=xtile.offset + off + 6,
                        ap=[pc, [9 * W, no], [7, 5], [W, 10], [1, 8]],
                    ),
                    axis=XY,
                )

            # --- steady tiles 0..10 -------------------------------------
            for i in range(nfull):
                tH = tp.tile([P, HO, W], f32)
                wout = wp.tile([P, HO * WO], f32)
                h_pool(xt[i], tH)
                w_pool(tH, wout)
                act_scales(wout, i * HO * WO, 0, HO)
                if i == 8:
                    # Batched store, tiles 0..7. Emitted here so its DMA
                    # request trails every load request; its data (scales
                    # 0..7) is already ordered on Act.
                    nc.scalar.dma_start(
                        out=bass.AP(
                            tensor=out,
                            offset=0,
                            ap=[[HO * WO, P], [HO * WO * P, 8], [1, HO * WO]],
                        ),
                        in_=bass.AP(
                            tensor=os_.tensor,
                            offset=os_.offset,
                            ap=[po, [HO * WO, 8], [1, HO * WO]],
                        ),
                    )

            # --- tile 10 as two chunks ----------------------------------
            wout10 = wp.tile([P, HO * WO], f32)
            tH10 = tp.tile([P, HO, W], f32)
            obase10 = nfull * HO * WO
            chunk_hw(xa10, tH10, wout10, 0, 6, 0)
            act_scales(wout10, obase10, 0, 6)
            fused_chunk(xc10, wout10, 6, 1, 54)
            act_scales(wout10, obase10, 6, 1)
            # --- tile 11 chunks -----------------------------------------
            wout11 = wp.tile([P, HO * WO], f32)
            tH11 = tp.tile([P, HO, W], f32)
            obase11 = (nfull + 1) * HO * WO
            chunk_hw(xa, tH11, wout11, 0, 3, 0, do_w=False)
            chunk_hw(xb, tH11, wout11, 3, 3, 27, do_w=False)
            pw11 = list(wout11.ap[0])
            ph11 = list(tH11.ap[0])
            nc.vector.reduce_sum(
                out=bass.AP(
                    tensor=wout11.tensor,
                    offset=wout11.offset,
                    ap=[pw11, [WO, 6], [6, 2]],
                ),
                in_=bass.AP(
                    tensor=tH11.tensor,
                    offset=tH11.offset,
                    ap=[ph11, [W, 6], [41, 2], [1, 7]],
                ),
                axis=X,
            )
            nc.vector.reduce_sum(
                out=bass.AP(
                    tensor=wout11.tensor,
                    offset=wout11.offset + 1,
                    ap=[pw11, [WO, 6], [1, 5]],
                ),
                in_=bass.AP(
                    tensor=tH11.tensor,
                    offset=tH11.offset + 6,
                    ap=[ph11, [W, 6], [7, 5], [1, 8]],
                ),
                axis=X,
            )
            act_scales(wout11, obase11, 0, 6)
            # tiles 8..10 full-block store once tile 10's scales are done
            nc.scalar.dma_start(
                out=bass.AP(
                    tensor=out,
                    offset=8 * HO * WO * P,
                    ap=[[HO * WO, P], [HO * WO * P, 3], [1, HO * WO]],
                ),
                in_=bass.AP(
                    tensor=os_.tensor,
                    offset=os_.offset + 8 * HO * WO,
                    ap=[po, [HO * WO, 3], [1, HO * WO]],
                ),
            )
            # o6: fused reduce + DVE scales + SP store (the tail chain)
            pc = list(xc.ap[0])
            pw = list(wout11.ap[0])
            nc.vector.reduce_sum(
                out=bass.AP(
                    tensor=wout11.tensor,
                    offset=wout11.offset + 42,
                    ap=[pw, [6, 2]],
                ),
                in_=bass.AP(
                    tensor=xc.tensor,
                    offset=xc.offset,
                    ap=[pc, [41, 2], [W, 10], [1, 7]],
                ),
                axis=XY,
            )
            nc.vector.reduce_sum(
                out=bass.AP(
                    tensor=wout11.tensor,
                    offset=wout11.offset + 43,
                    ap=[pw, [1, 5]],
                ),
                in_=bass.AP(
                    tensor=xc.tensor,
                    offset=xc.offset + 6,
                    ap=[pc, [7, 5], [W, 10], [1, 8]],
                ),
                axis=XY,
            )
            nc.vector.tensor_scalar_mul(
                bass.AP(
                    tensor=os_.tensor,
                    offset=os_.offset + obase11 + 42,
                    ap=[po, [6, 2]],
                ),
                bass.AP(
                    tensor=wout11.tensor,
                    offset=wout11.offset + 42,
                    ap=[pw, [6, 2]],
                ),
                1.0 / 70.0,
            )
            nc.vector.tensor_scalar_mul(
                bass.AP(
                    tensor=os_.tensor,
                    offset=os_.offset + obase11 + 43,
                    ap=[po, [1, 5]],
                ),
                bass.AP(
                    tensor=wout11.tensor,
                    offset=wout11.offset + 43,
                    ap=[pw, [1, 5]],
                ),
                1.0 / 80.0,
            )
            nc.sync.dma_start(
                out=out[(nfull + 1) * P :, :],
                in_=os_[:, obase11 : obase11 + HO * WO],
            )
    _strip_pool_ring_memsets(nc)
    _legalize_multiwait(nc)
    _hoist_first_load(nc)
    return nc


def kernel(x: np.ndarray) -> np.ndarray:
    global _nc_cache
    from concourse.bass_utils import run_bass_kernel_spmd

    xr = np.ascontiguousarray(
        np.asarray(x, dtype=np.float32).reshape(B * C, H * W)
    )
    if _nc_cache is None:
        _nc_cache = _build()
    nc = _nc_cache
    in_maps = [{"x": xr[k * ROWS : (k + 1) * ROWS]} for k in range(NCORES)]
    res = run_bass_kernel_spmd(nc, in_maps, list(range(NCORES)))
    out = np.concatenate([r["out"] for r in res.results], axis=0)
    return out.reshape(B, C, HO, WO)


# revision 6
# speedup vs baseline: 1.0009x; 1.0009x over previous
"""Adaptive avg pool 2D (16,768,64,48) -> (16,768,7,7) on 8 TRN2 NeuronCores.

Data-parallel over B*C rows: 1536 rows/core, 12 tiles of 128 rows. The DMA
device is the bottleneck (52.4us of loads at the modeled 360B/ns), so the
kernel is built around a gap-free load stream plus a minimal post-load tail:

- The first load's dma_start is hoisted into the SP preamble (before the
  entry barrier) so the DMA device starts at ~1.55us instead of ~2.3us.
- Loads stay alone on the 8 HWDGE issue lanes (a HWDGE DMA's issue waits
  its lane predecessor's completion, so a store in the rotation would stall
  later loads), and every store is emitted late enough that its DMA-FIFO
  request trails the last load request.
- Steady tiles 0..9: H-pool (all windows 10 rows, stride 9) split across
  engines - DVE windowed reduce for w in [0,39), Pool as a 9-add chain for
  w in [39,48); W-pool as two DVE reduces (q in {0,6}: size-7 windows,
  q in 1..5: size-8); scales 1/70, 1/80 on Act into a [128, 588] staging
  tile. Completion trails each tile's load by ~4.3us.
- Tile 10 loads as o0-5 + o6 h-chunks, tile 11 as o0-2 / o3-5 / o6, so the
  compute after the final 683ns o6 load is just one fused (h,w) reduce
  pair plus two tiny DVE scales.
- Stores: tiles 0-7 and 8-10 as two batched Act DMAs (explicit 3-dim DRAM
  APs - row = 128*tile + partition), tile 11 as one SP DMA at the very end.
- A post-pass drops the unused SWDGE-ring preamble memsets (no gpsimd
  DMAs here), another hoists the first load, and _legalize_multiwait
  splits multi-wait sync_infos for this walrus (max 1 wait/instruction),
  ordering the final store's lane sem last so the end drain does not
  trail it with already-satisfied waits.

Cost-model timeline: 60444 ns/core (baseline 62571; HBM floor ~52.6us).
"""

import sys

_TRN_REPO = "/opt/trn_rl_repo"
if _TRN_REPO not in sys.path:
    sys.path.insert(0, _TRN_REPO)

import numpy as np

import concourse.bass as bass
import concourse.mybir as mybir
from concourse.tile import TileContext

B, C, H, W = 16, 768, 64, 48
HO, WO = 7, 7
NCORES = 8
ROWS = B * C // NCORES  # 1536
P = 128
NTILES = ROWS // P  # 12
WD = 39  # steady
WDB = 40  # tile-11 chunk-B DVE H cols (lighter Pool share): DVE H cols [0, WD), Pool cols [WD, 48)
f32 = mybir.dt.float32
X = mybir.AxisListType.X
XY = mybir.AxisListType.XY

_nc_cache = None


def _legalize_multiwait(nc: bass.Bass) -> None:
    """Walrus accepts at most one sync wait per instruction (two for
    EventSemaphore). Hoist extra waits into single-wait EventSemaphore
    carriers placed directly before the offending instruction."""
    n = 0
    final_lane = None
    for b in nc.m.functions[0].blocks:
        for inst in b.instructions:
            if type(inst).__name__ == "InstDMACopy" and inst.sync_info:
                for w in inst.sync_info.on_wait:
                    if (w.ant_name or "").startswith("DMAHW"):
                        final_lane = w.ant_name
    for b in nc.m.functions[0].blocks:
        insts = b.instructions
        i = 0
        while i < len(insts):
            inst = insts[i]
            si = inst.sync_info
            if si is not None and len(si.on_wait) > 1:
                waits = sorted(
                    si.on_wait,
                    key=lambda w: (
                        2
                        if w.ant_name == final_lane
                        else 1
                        if (w.ant_name or "").startswith("DMA")
                        else 0
                    ),
                )
                carriers = []
                rest = waits[:-1]
                for j in range(0, len(rest), 2):
                    n += 1
                    ev = mybir.InstEventSemaphore(
                        name=f"I-waitfix-{n}", ins=[], outs=[]
                    )
                    ev.engine = inst.engine
                    ev.sync_info = mybir.SyncInfo(
                        on_wait=rest[j : j + 2], on_update=[]
                    )
                    nc.register_instruction(ev)
                    carriers.append(ev)
                inst.sync_info = mybir.SyncInfo(
                    on_wait=[waits[-1]], on_update=list(si.on_update)
                )
                insts[i:i] = carriers
                i += len(carriers)
            i += 1



def _strip_pool_ring_memsets(nc: bass.Bass) -> None:
    """The framework preamble memsets the SWDGE descriptor rings on Pool.
    This kernel issues no SWDGE DMAs (loads ride SP, stores Act/SP via
    HWDGE), so the ring init only delays the entry barrier; drop it."""
    for b in nc.m.functions[0].blocks:
        keep = [
            i
            for i in b.instructions
            if not (
                type(i).__name__ == "InstMemset"
                and str(i.engine).endswith("Pool")
                and i.name in ("I-29", "I-30", "I-31", "I-32")
            )
        ]
        if len(keep) != len(b.instructions):
            b.instructions[:] = keep



def _hoist_first_load(nc: bass.Bass) -> None:
    """Move the first load's dma_start into the preamble block, right after
    SP's queue-setup RegisterMoves and before the entry barrier. The load
    has no waits and its consumers are semaphore-gated, so it can issue
    while the other engines are still running their preambles (~0.5us
    earlier DMA start)."""
    blocks = nc.m.functions[0].blocks
    if len(blocks) < 2:
        return
    b0, b1 = blocks[0], blocks[1]
    first = None
    for i in b1.instructions:
        if type(i).__name__ == "InstDMACopy" and str(i.engine).endswith("SP"):
            if i.sync_info and i.sync_info.on_wait:
                return  # unexpected: keep conservative
            first = i
            break
    if first is None:
        return
    b1.instructions.remove(first)
    idx = next(
        (k for k, i in enumerate(b0.instructions) if type(i).__name__ == "InstDrain"),
        None,
    )
    if idx is None:
        b1.instructions.insert(0, first)
        return
    b0.instructions.insert(idx, first)


def _build() -> bass.Bass:
    nc = bass.Bass()
    x = nc.dram_tensor("x", [ROWS, H * W], f32, kind="ExternalInput")
    out = nc.dram_tensor("out", [ROWS, HO * WO], f32, kind="ExternalOutput")
    ACopy = mybir.ActivationFunctionType.Copy
    with TileContext(nc) as tc:
        with (
            tc.tile_pool(name="xp", bufs=NTILES - 2) as xp,
            tc.tile_pool(name="cp", bufs=1) as cpool,
            tc.tile_pool(name="tp", bufs=3) as tp,
            tc.tile_pool(name="wp", bufs=3) as wp,
            tc.tile_pool(name="op", bufs=1) as op,
        ):
            nfull = NTILES - 2
            os_ = op.tile([P, NTILES * HO * WO], f32)  # [128, 588] staging
            po = list(os_.ap[0])

            # --- loads: 10 full tiles; tile 10 as o0-5 + o6 h-chunks;
            #     tile 11 as o0-2 / o3-5 / o6 h-chunks ---------------------
            xt = []
            for i in range(nfull):
                t = xp.tile([P, H, W], f32)
                nc.sync.dma_start(
                    out=t,
                    in_=x[i * P : (i + 1) * P, :].rearrange(
                        "p (h w) -> p h w", w=W
                    ),
                )
                xt.append(t)
            rows10 = x[nfull * P : (nfull + 1) * P, :].rearrange(
                "p (h w) -> p h w", w=W
            )
            xa10 = cpool.tile([P, 55, W], f32, tag="xa10")
            nc.sync.dma_start(out=xa10, in_=rows10[:, 0:55, :])
            xc10 = cpool.tile([P, 10, W], f32, tag="xc10")
            nc.sync.dma_start(out=xc10, in_=rows10[:, 54:64, :])
            rows11 = x[(nfull + 1) * P :, :].rearrange("p (h w) -> p h w", w=W)
            xa = cpool.tile([P, 28, W], f32, tag="xa")
            nc.sync.dma_start(out=xa, in_=rows11[:, 0:28, :])
            xb = cpool.tile([P, 28, W], f32, tag="xb")
            nc.sync.dma_start(out=xb, in_=rows11[:, 27:55, :])
            xc = cpool.tile([P, 10, W], f32, tag="xc")
            nc.sync.dma_start(out=xc, in_=rows11[:, 54:64, :])

            # --- helpers -------------------------------------------------
            def h_pool(xtile, tH):
                pt = list(xtile.ap[0])
                ph = list(tH.ap[0])
                nc.vector.reduce_sum(
                    out=bass.AP(
                        tensor=tH.tensor,
                        offset=tH.offset,
                        ap=[ph, [W, HO], [1, WD]],
                    ),
                    in_=bass.AP(
                        tensor=xtile.tensor,
                        offset=xtile.offset,
                        ap=[pt, [9 * W, HO], [1, WD], [W, 10]],
                    ),
                    axis=X,
                )
                tsl = bass.AP(
                    tensor=tH.tensor,
                    offset=tH.offset + WD,
                    ap=[ph, [W, HO], [1, W - WD]],
                )

                def xsl(k):
                    return bass.AP(
                        tensor=xtile.tensor,
                        offset=xtile.offset + k * W + WD,
                        ap=[pt, [9 * W, HO], [1, W - WD]],
                    )

                nc.gpsimd.tensor_add(tsl, xsl(0), xsl(1))
                for k in range(2, 10):
                    nc.gpsimd.tensor_add(tsl, tsl, xsl(k))

            def w_pool(tH, wout):
                ph = list(tH.ap[0])
                pw = list(wout.ap[0])
                nc.vector.reduce_sum(
                    out=bass.AP(
                        tensor=wout.tensor,
                        offset=wout.offset,
                        ap=[pw, [WO, HO], [6, 2]],
                    ),
                    in_=bass.AP(
                        tensor=tH.tensor,
                        offset=tH.offset,
                        ap=[ph, [W, HO], [41, 2], [1, 7]],
                    ),
                    axis=X,
                )
                nc.vector.reduce_sum(
                    out=bass.AP(
                        tensor=wout.tensor,
                        offset=wout.offset + 1,
                        ap=[pw, [WO, HO], [1, 5]],
                    ),
                    in_=bass.AP(
                        tensor=tH.tensor,
                        offset=tH.offset + 6,
                        ap=[ph, [W, HO], [7, 5], [1, 8]],
                    ),
                    axis=X,
                )

            def act_scales(wout, obase, o0, no):
                pw = list(wout.ap[0])
                nc.scalar.activation(
                    out=bass.AP(
                        tensor=os_.tensor,
                        offset=os_.offset + obase + 7 * o0,
                        ap=[po, [WO, no], [6, 2]],
                    ),
                    in_=bass.AP(
                        tensor=wout.tensor,
                        offset=wout.offset + 7 * o0,
                        ap=[pw, [WO, no], [6, 2]],
                    ),
                    func=ACopy,
                    scale=1.0 / 70.0,
                )
                nc.scalar.activation(
                    out=bass.AP(
                        tensor=os_.tensor,
                        offset=os_.offset + obase + 7 * o0 + 1,
                        ap=[po, [WO, no], [1, 5]],
                    ),
                    in_=bass.AP(
                        tensor=wout.tensor,
                        offset=wout.offset + 7 * o0 + 1,
                        ap=[pw, [WO, no], [1, 5]],
                    ),
                    func=ACopy,
                    scale=1.0 / 80.0,
                )

            def chunk_hw(xtile, tH, wout, o0, no, h0, do_w=True, wd=None):
                """H-pool rows [o0,o0+no) split DVE/Pool, then W-pool."""
                wd = WD if wd is None else wd
                pt = list(xtile.ap[0])
                ph = list(tH.ap[0])
                pw = list(wout.ap[0])
                off = (9 * o0 - h0) * W
                nc.vector.reduce_sum(
                    out=bass.AP(
                        tensor=tH.tensor,
                        offset=tH.offset + o0 * W,
                        ap=[ph, [W, no], [1, wd]],
                    ),
                    in_=bass.AP(
                        tensor=xtile.tensor,
                        offset=xtile.offset + off,
                        ap=[pt, [9 * W, no], [1, wd], [W, 10]],
                    ),
                    axis=X,
                )
                tsl = bass.AP(
                    tensor=tH.tensor,
                    offset=tH.offset + o0 * W + wd,
                    ap=[ph, [W, no], [1, W - wd]],
                )

                def xsl(k):
                    return bass.AP(
                        tensor=xtile.tensor,
                        offset=xtile.offset + off + k * W + wd,
                        ap=[pt, [9 * W, no], [1, W - wd]],
                    )

                nc.gpsimd.tensor_add(tsl, xsl(0), xsl(1))
                for k in range(2, 10):
                    nc.gpsimd.tensor_add(tsl, tsl, xsl(k))
                if not do_w:
                    return
                nc.vector.reduce_sum(
                    out=bass.AP(
                        tensor=wout.tensor,
                        offset=wout.offset + 7 * o0,
                        ap=[pw, [WO, no], [6, 2]],
                    ),
                    in_=bass.AP(
                        tensor=tH.tensor,
                        offset=tH.offset + o0 * W,
                        ap=[ph, [W, no], [41, 2], [1, 7]],
                    ),
                    axis=X,
                )
                nc.vector.reduce_sum(
                    out=bass.AP(
                        tensor=wout.tensor,
                        offset=wout.offset + 7 * o0 + 1,
                        ap=[pw, [WO, no], [1, 5]],
                    ),
                    in_=bass.AP(
                        tensor=tH.tensor,
                        offset=tH.offset + o0 * W + 6,
                        ap=[ph, [W, no], [7, 5], [1, 8]],
                    ),
                    axis=X,
                )

            def fused_chunk(xtile, wout, o0, no, h0):
                """wout[:, 7o+q] for o in [o0,o0+no) by 2D (h,w) reduces."""
                pc = list(xtile.ap[0])
                pw = list(wout.ap[0])
                off = (9 * o0 - h0) * W
                nc.vector.reduce_sum(
                    out=bass.AP(
                        tensor=wout.tensor,
                        offset=wout.offset + 7 * o0,
                        ap=[pw, [WO, no], [6, 2]],
                    ),
                    in_=bass.AP(
                        tensor=xtile.tensor,
                        offset=xtile.offset + off,
                        ap=[pc, [9 * W, no], [41, 2], [W, 10], [1, 7]],
                    ),
                    axis=XY,
                )
                nc.vector.reduce_sum(
                    out=bass.AP(
                        tensor=wout.tensor,
                        offset=wout.offset + 7 * o0 + 1,
                        ap=[pw, [WO, no], [1, 5]],
                    ),
                    in_=bass.AP(
                        tensor=xtile.tensor,
                        offset=xtile.offset + off + 6,
                        ap=[pc, [9 * W, no], [7, 5], [W, 10], [1, 8]],
                    ),
                    axis=XY,
                )

            # --- steady tiles 0..10 -------------------------------------
            for i in range(nfull):
                tH = tp.tile([P, HO, W], f32)
                wout = wp.tile([P, HO * WO], f32)
                h_pool(xt[i], tH)
                w_pool(tH, wout)
                act_scales(wout, i * HO * WO, 0, HO)
                if i == 8:
                    # Batched store, tiles 0..7. Emitted here so its DMA
                    # request trails every load request; its data (scales
                    # 0..7) is already ordered on Act.
                    nc.scalar.dma_start(
                        out=bass.AP(
                            tensor=out,
                            offset=0,
                            ap=[[HO * WO, P], [HO * WO * P, 8], [1, HO * WO]],
                        ),
                        in_=bass.AP(
                            tensor=os_.tensor,
                            offset=os_.offset,
                            ap=[po, [HO * WO, 8], [1, HO * WO]],
                        ),
                    )

            # --- tile 10 as two chunks ----------------------------------
            wout10 = wp.tile([P, HO * WO], f32)
            tH10 = tp.tile([P, HO, W], f32)
            obase10 = nfull * HO * WO
            chunk_hw(xa10, tH10, wout10, 0, 6, 0)
            act_scales(wout10, obase10, 0, 6)
            fused_chunk(xc10, wout10, 6, 1, 54)
            act_scales(wout10, obase10, 6, 1)
            # --- tile 11 chunks -----------------------------------------
            wout11 = wp.tile([P, HO * WO], f32)
            tH11 = tp.tile([P, HO, W], f32)
            obase11 = (nfull + 1) * HO * WO
            chunk_hw(xa, tH11, wout11, 0, 3, 0, do_w=False)
            chunk_hw(xb, tH11, wout11, 3, 3, 27, do_w=False, wd=WDB)
            pw11 = list(wout11.ap[0])
            ph11 = list(tH11.ap[0])
            nc.vector.reduce_sum(
                out=bass.AP(
                    tensor=wout11.tensor,
                    offset=wout11.offset,
                    ap=[pw11, [WO, 6], [6, 2]],
                ),
                in_=bass.AP(
                    tensor=tH11.tensor,
                    offset=tH11.offset,
                    ap=[ph11, [W, 6], [41, 2], [1, 7]],
                ),
                axis=X,
            )
            nc.vector.reduce_sum(
                out=bass.AP(
                    tensor=wout11.tensor,
                    offset=wout11.offset + 1,
                    ap=[pw11, [WO, 6], [1, 5]],
                ),
                in_=bass.AP(
                    tensor=tH11.tensor,
                    offset=tH11.offset + 6,
                    ap=[ph11, [W, 6], [7, 5], [1, 8]],
                ),
                axis=X,
            )
            act_scales(wout11, obase11, 0, 6)
            # tiles 8..10 full-block store once tile 10's scales are done
            nc.scalar.dma_start(
                out=bass.AP(
                    tensor=out,
                    offset=8 * HO * WO * P,
                    ap=[[HO * WO, P], [HO * WO * P, 3], [1, HO * WO]],
                ),
                in_=bass.AP(
                    tensor=os_.tensor,
                    offset=os_.offset + 8 * HO * WO,
                    ap=[po, [HO * WO, 3], [1, HO * WO]],
                ),
            )
            # o6: fused reduce + DVE scales + SP store (the tail chain)
            pc = list(xc.ap[0])
            pw = list(wout11.ap[0])
            nc.vector.reduce_sum(
                out=bass.AP(
                    tensor=wout11.tensor,
                    offset=wout11.offset + 42,
                    ap=[pw, [6, 2]],
                ),
                in_=bass.AP(
                    tensor=xc.tensor,
                    offset=xc.offset,
                    ap=[pc, [41, 2], [W, 10], [1, 7]],
                ),
                axis=XY,
            )
            nc.vector.reduce_sum(
                out=bass.AP(
                    tensor=wout11.tensor,
                    offset=wout11.offset + 43,
                    ap=[pw, [1, 5]],
                ),
                in_=bass.AP(
                    tensor=xc.tensor,
                    offset=xc.offset + 6,
                    ap=[pc, [7, 5], [W, 10], [1, 8]],
                ),
                axis=XY,
            )
            nc.vector.tensor_scalar_mul(
                bass.AP(
                    tensor=os_.tensor,
                    offset=os_.offset + obase11 + 42,
                    ap=[po, [6, 2]],
                ),
                bass.AP(
                    tensor=wout11.tensor,
                    offset=wout11.offset + 42,
                    ap=[pw, [6, 2]],
                ),
                1.0 / 70.0,
            )
            nc.vector.tensor_scalar_mul(
                bass.AP(
                    tensor=os_.tensor,
                    offset=os_.offset + obase11 + 43,
                    ap=[po, [1, 5]],
                ),
                bass.AP(
                    tensor=wout11.tensor,
                    offset=wout11.offset + 43,
                    ap=[pw, [1, 5]],
                ),
                1.0 / 80.0,
            )
            nc.sync.dma_start(
                out=out[(nfull + 1) * P :, :],
                in_=os_[:, obase11 : obase11 + HO * WO],
            )
    _strip_pool_ring_memsets(nc)
    _legalize_multiwait(nc)
    _hoist_first_load(nc)
    return nc


def kernel(x: np.ndarray) -> np.ndarray:
    global _nc_cache
    from concourse.bass_utils import run_bass_kernel_spmd

    xr = np.ascontiguousarray(
        np.asarray(x, dtype=np.float32).reshape(B * C, H * W)
    )
    if _nc_cache is None:
        _nc_cache = _build()
    nc = _nc_cache
    in_maps = [{"x": xr[k * ROWS : (k + 1) * ROWS]} for k in range(NCORES)]
    res = run_bass_kernel_spmd(nc, in_maps, list(range(NCORES)))
    out = np.concatenate([r["out"] for r in res.results], axis=0)
    return out.reshape(B, C, HO, WO)


# revision 7
# speedup vs baseline: 1.0052x; 1.0043x over previous
"""Adaptive avg pool 2D (16,768,64,48) -> (16,768,7,7) on 8 TRN2 NeuronCores.

Data-parallel over B*C rows: 1536 rows/core, 12 tiles of 128 rows. The DMA
device is the bottleneck (52.4us of loads at the modeled 360B/ns), so the
kernel is built around a gap-free load stream plus a minimal post-load tail:

- The first load's dma_start is hoisted into the SP preamble (before the
  entry barrier) so the DMA device starts at ~1.55us instead of ~2.3us.
- Loads stay alone on the 8 HWDGE issue lanes (a HWDGE DMA's issue waits
  its lane predecessor's completion, so a store in the rotation would stall
  later loads), and every store is emitted late enough that its DMA-FIFO
  request trails the last load request.
- Steady tiles 0..9: H-pool (all windows 10 rows, stride 9) split across
  engines - DVE windowed reduce for w in [0,39), Pool as a 9-add chain for
  w in [39,48); W-pool as two DVE reduces (q in {0,6}: size-7 windows,
  q in 1..5: size-8); scales 1/70, 1/80 on Act into a [128, 588] staging
  tile. Completion trails each tile's load by ~4.3us.
- Tile 10 loads as o0-5 + o6 h-chunks, tile 11 as o0-2 / o3-5 / o6, so the
  compute after the final 683ns o6 load is just one fused (h,w) reduce
  pair plus two tiny DVE scales.
- Stores: tiles 0-7 and 8-10 as two batched Act DMAs (explicit 3-dim DRAM
  APs - row = 128*tile + partition), tile 11 as one SP DMA at the very end.
- A post-pass drops the unused SWDGE-ring preamble memsets (no gpsimd
  DMAs here), another hoists the first load, and _legalize_multiwait
  splits multi-wait sync_infos for this walrus (max 1 wait/instruction),
  ordering the final store's lane sem last so the end drain does not
  trail it with already-satisfied waits.

Cost-model timeline: 60444 ns/core (baseline 62571; HBM floor ~52.6us).
"""

import sys

_TRN_REPO = "/opt/trn_rl_repo"
if _TRN_REPO not in sys.path:
    sys.path.insert(0, _TRN_REPO)

import numpy as np

import concourse.bass as bass
import concourse.mybir as mybir
from concourse.tile import TileContext

B, C, H, W = 16, 768, 64, 48
HO, WO = 7, 7
NCORES = 8
ROWS = B * C // NCORES  # 1536
P = 128
NTILES = ROWS // P  # 12
WD = 39  # steady
WDB = 40  # tile-11 chunk-B DVE H cols (lighter Pool share): DVE H cols [0, WD), Pool cols [WD, 48)
f32 = mybir.dt.float32
X = mybir.AxisListType.X
XY = mybir.AxisListType.XY

_nc_cache = None


def _legalize_multiwait(nc: bass.Bass) -> None:
    """Walrus accepts at most one sync wait per instruction (two for
    EventSemaphore). Hoist extra waits into single-wait EventSemaphore
    carriers placed directly before the offending instruction."""
    n = 0
    final_lane = None
    for b in nc.m.functions[0].blocks:
        for inst in b.instructions:
            if type(inst).__name__ == "InstDMACopy" and inst.sync_info:
                for w in inst.sync_info.on_wait:
                    if (w.ant_name or "").startswith("DMAHW"):
                        final_lane = w.ant_name
    for b in nc.m.functions[0].blocks:
        insts = b.instructions
        i = 0
        while i < len(insts):
            inst = insts[i]
            si = inst.sync_info
            if si is not None and len(si.on_wait) > 1:
                waits = sorted(
                    si.on_wait,
                    key=lambda w: (
                        2
                        if w.ant_name == final_lane
                        else 1
                        if (w.ant_name or "").startswith("DMA")
                        else 0
                    ),
                )
                carriers = []
                rest = waits[:-1]
                for j in range(0, len(rest), 2):
                    n += 1
                    ev = mybir.InstEventSemaphore(
                        name=f"I-waitfix-{n}", ins=[], outs=[]
                    )
                    ev.engine = inst.engine
                    ev.sync_info = mybir.SyncInfo(
                        on_wait=rest[j : j + 2], on_update=[]
                    )
                    nc.register_instruction(ev)
                    carriers.append(ev)
                inst.sync_info = mybir.SyncInfo(
                    on_wait=[waits[-1]], on_update=list(si.on_update)
                )
                insts[i:i] = carriers
                i += len(carriers)
            i += 1



def _strip_pool_ring_memsets(nc: bass.Bass) -> None:
    """The framework preamble memsets the SWDGE descriptor rings on Pool.
    This kernel issues no SWDGE DMAs (loads ride SP, stores Act/SP via
    HWDGE), so the ring init only delays the entry barrier; drop it."""
    for b in nc.m.functions[0].blocks:
        keep = [
            i
            for i in b.instructions
            if not (
                type(i).__name__ == "InstMemset"
                and str(i.engine).endswith("Pool")
                and i.name in ("I-29", "I-30", "I-31", "I-32")
            )
        ]
        if len(keep) != len(b.instructions):
            b.instructions[:] = keep



def _hoist_first_load(nc: bass.Bass) -> None:
    """Move the first load's dma_start into the preamble block, right after
    SP's queue-setup RegisterMoves and before the entry barrier. The load
    has no waits and its consumers are semaphore-gated, so it can issue
    while the other engines are still running their preambles (~0.5us
    earlier DMA start)."""
    blocks = nc.m.functions[0].blocks
    if len(blocks) < 2:
        return
    b0, b1 = blocks[0], blocks[1]
    first = None
    for i in b1.instructions:
        if type(i).__name__ == "InstDMACopy" and str(i.engine).endswith("SP"):
            if i.sync_info and i.sync_info.on_wait:
                return  # unexpected: keep conservative
            first = i
            break
    if first is None:
        return
    b1.instructions.remove(first)
    idx = next(
        (k for k, i in enumerate(b0.instructions) if type(i).__name__ == "InstDrain"),
        None,
    )
    if idx is None:
        b1.instructions.insert(0, first)
        return
    b0.instructions.insert(idx, first)



def _strip_final_barrier_round(nc: bass.Bass) -> None:
    """The module tail ends with two full all-engine barrier rounds around
    the Pool dge-drain ISA. The second round's waits are already satisfied
    when it runs (same gather/release thresholds as round one), so it only
    adds ~300ns of decode/propagation latency. Drop everything after the
    final Pool ISA; round one (which carries the real DMA-completion
    waits) remains the kernel end."""
    blocks = nc.m.functions[0].blocks
    lastb = blocks[-1]
    isa_idx = None
    for k, i in enumerate(lastb.instructions):
        if type(i).__name__ == "InstISA":
            isa_idx = k
    if isa_idx is not None:
        del lastb.instructions[isa_idx + 1 :]


def _build() -> bass.Bass:
    nc = bass.Bass()
    x = nc.dram_tensor("x", [ROWS, H * W], f32, kind="ExternalInput")
    out = nc.dram_tensor("out", [ROWS, HO * WO], f32, kind="ExternalOutput")
    ACopy = mybir.ActivationFunctionType.Copy
    with TileContext(nc) as tc:
        with (
            tc.tile_pool(name="xp", bufs=NTILES - 2) as xp,
            tc.tile_pool(name="cp", bufs=1) as cpool,
            tc.tile_pool(name="tp", bufs=3) as tp,
            tc.tile_pool(name="wp", bufs=3) as wp,
            tc.tile_pool(name="op", bufs=1) as op,
        ):
            nfull = NTILES - 2
            os_ = op.tile([P, NTILES * HO * WO], f32)  # [128, 588] staging
            po = list(os_.ap[0])

            # --- loads: 10 full tiles; tile 10 as o0-5 + o6 h-chunks;
            #     tile 11 as o0-2 / o3-5 / o6 h-chunks ---------------------
            xt = []
            for i in range(nfull):
                t = xp.tile([P, H, W], f32)
                nc.sync.dma_start(
                    out=t,
                    in_=x[i * P : (i + 1) * P, :].rearrange(
                        "p (h w) -> p h w", w=W
                    ),
                )
                xt.append(t)
            rows10 = x[nfull * P : (nfull + 1) * P, :].rearrange(
                "p (h w) -> p h w", w=W
            )
            xa10 = cpool.tile([P, 55, W], f32, tag="xa10")
            nc.sync.dma_start(out=xa10, in_=rows10[:, 0:55, :])
            xc10 = cpool.tile([P, 10, W], f32, tag="xc10")
            nc.sync.dma_start(out=xc10, in_=rows10[:, 54:64, :])
            rows11 = x[(nfull + 1) * P :, :].rearrange("p (h w) -> p h w", w=W)
            xa = cpool.tile([P, 28, W], f32, tag="xa")
            nc.sync.dma_start(out=xa, in_=rows11[:, 0:28, :])
            xb = cpool.tile([P, 28, W], f32, tag="xb")
            nc.sync.dma_start(out=xb, in_=rows11[:, 27:55, :])
            xc = cpool.tile([P, 10, W], f32, tag="xc")
            nc.sync.dma_start(out=xc, in_=rows11[:, 54:64, :])

            # --- helpers -------------------------------------------------
            def h_pool(xtile, tH):
                pt = list(xtile.ap[0])
                ph = list(tH.ap[0])
                nc.vector.reduce_sum(
                    out=bass.AP(
                        tensor=tH.tensor,
                        offset=tH.offset,
                        ap=[ph, [W, HO], [1, WD]],
                    ),
                    in_=bass.AP(
                        tensor=xtile.tensor,
                        offset=xtile.offset,
                        ap=[pt, [9 * W, HO], [1, WD], [W, 10]],
                    ),
                    axis=X,
                )
                tsl = bass.AP(
                    tensor=tH.tensor,
                    offset=tH.offset + WD,
                    ap=[ph, [W, HO], [1, W - WD]],
                )

                def xsl(k):
                    return bass.AP(
                        tensor=xtile.tensor,
                        offset=xtile.offset + k * W + WD,
                        ap=[pt, [9 * W, HO], [1, W - WD]],
                    )

                nc.gpsimd.tensor_add(tsl, xsl(0), xsl(1))
                for k in range(2, 10):
                    nc.gpsimd.tensor_add(tsl, tsl, xsl(k))

            def w_pool(tH, wout):
                ph = list(tH.ap[0])
                pw = list(wout.ap[0])
                nc.vector.reduce_sum(
                    out=bass.AP(
                        tensor=wout.tensor,
                        offset=wout.offset,
                        ap=[pw, [WO, HO], [6, 2]],
                    ),
                    in_=bass.AP(
                        tensor=tH.tensor,
                        offset=tH.offset,
                        ap=[ph, [W, HO], [41, 2], [1, 7]],
                    ),
                    axis=X,
                )
                nc.vector.reduce_sum(
                    out=bass.AP(
                        tensor=wout.tensor,
                        offset=wout.offset + 1,
                        ap=[pw, [WO, HO], [1, 5]],
                    ),
                    in_=bass.AP(
                        tensor=tH.tensor,
                        offset=tH.offset + 6,
                        ap=[ph, [W, HO], [7, 5], [1, 8]],
                    ),
                    axis=X,
                )

            def act_scales(wout, obase, o0, no):
                pw = list(wout.ap[0])
                nc.scalar.activation(
                    out=bass.AP(
                        tensor=os_.tensor,
                        offset=os_.offset + obase + 7 * o0,
                        ap=[po, [WO, no], [6, 2]],
                    ),
                    in_=bass.AP(
                        tensor=wout.tensor,
                        offset=wout.offset + 7 * o0,
                        ap=[pw, [WO, no], [6, 2]],
                    ),
                    func=ACopy,
                    scale=1.0 / 70.0,
                )
                nc.scalar.activation(
                    out=bass.AP(
                        tensor=os_.tensor,
                        offset=os_.offset + obase + 7 * o0 + 1,
                        ap=[po, [WO, no], [1, 5]],
                    ),
                    in_=bass.AP(
                        tensor=wout.tensor,
                        offset=wout.offset + 7 * o0 + 1,
                        ap=[pw, [WO, no], [1, 5]],
                    ),
                    func=ACopy,
                    scale=1.0 / 80.0,
                )

            def chunk_hw(xtile, tH, wout, o0, no, h0, do_w=True, wd=None):
                """H-pool rows [o0,o0+no) split DVE/Pool, then W-pool."""
                wd = WD if wd is None else wd
                pt = list(xtile.ap[0])
                ph = list(tH.ap[0])
                pw = list(wout.ap[0])
                off = (9 * o0 - h0) * W
                nc.vector.reduce_sum(
                    out=bass.AP(
                        tensor=tH.tensor,
                        offset=tH.offset + o0 * W,
                        ap=[ph, [W, no], [1, wd]],
                    ),
                    in_=bass.AP(
                        tensor=xtile.tensor,
                        offset=xtile.offset + off,
                        ap=[pt, [9 * W, no], [1, wd], [W, 10]],
                    ),
                    axis=X,
                )
                tsl = bass.AP(
                    tensor=tH.tensor,
                    offset=tH.offset + o0 * W + wd,
                    ap=[ph, [W, no], [1, W - wd]],
                )

                def xsl(k):
                    return bass.AP(
                        tensor=xtile.tensor,
                        offset=xtile.offset + off + k * W + wd,
                        ap=[pt, [9 * W, no], [1, W - wd]],
                    )

                nc.gpsimd.tensor_add(tsl, xsl(0), xsl(1))
                for k in range(2, 10):
                    nc.gpsimd.tensor_add(tsl, tsl, xsl(k))
                if not do_w:
                    return
                nc.vector.reduce_sum(
                    out=bass.AP(
                        tensor=wout.tensor,
                        offset=wout.offset + 7 * o0,
                        ap=[pw, [WO, no], [6, 2]],
                    ),
                    in_=bass.AP(
                        tensor=tH.tensor,
                        offset=tH.offset + o0 * W,
                        ap=[ph, [W, no], [41, 2], [1, 7]],
                    ),
                    axis=X,
                )
                nc.vector.reduce_sum(
                    out=bass.AP(
                        tensor=wout.tensor,
                        offset=wout.offset + 7 * o0 + 1,
                        ap=[pw, [WO, no], [1, 5]],
                    ),
                    in_=bass.AP(
                        tensor=tH.tensor,
                        offset=tH.offset + o0 * W + 6,
                        ap=[ph, [W, no], [7, 5], [1, 8]],
                    ),
                    axis=X,
                )

            def fused_chunk(xtile, wout, o0, no, h0):
                """wout[:, 7o+q] for o in [o0,o0+no) by 2D (h,w) reduces."""
                pc = list(xtile.ap[0])
                pw = list(wout.ap[0])
                off = (9 * o0 - h0) * W
                nc.vector.reduce_sum(
                    out=bass.AP(
                        tensor=wout.tensor,
                        offset=wout.offset + 7 * o0,
                        ap=[pw, [WO, no], [6, 2]],
                    ),
                    in_=bass.AP(
                        tensor=xtile.tensor,
                        offset=xtile.offset + off,
                        ap=[pc, [9 * W, no], [41, 2], [W, 10], [1, 7]],
                    ),
                    axis=XY,
                )
                nc.vector.reduce_sum(
                    out=bass.AP(
                        tensor=wout.tensor,
                        offset=wout.offset + 7 * o0 + 1,
                        ap=[pw, [WO, no], [1, 5]],
                    ),
                    in_=bass.AP(
                        tensor=xtile.tensor,
                        offset=xtile.offset + off + 6,
                        ap=[pc, [9 * W, no], [7, 5], [W, 10], [1, 8]],
                    ),
                    axis=XY,
                )

            # --- steady tiles 0..10 -------------------------------------
            for i in range(nfull):
                tH = tp.tile([P, HO, W], f32)
                wout = wp.tile([P, HO * WO], f32)
                h_pool(xt[i], tH)
                w_pool(tH, wout)
                act_scales(wout, i * HO * WO, 0, HO)
                if i == 8:
                    # Batched store, tiles 0..7. Emitted here so its DMA
                    # request trails every load request; its data (scales
                    # 0..7) is already ordered on Act.
                    nc.scalar.dma_start(
                        out=bass.AP(
                            tensor=out,
                            offset=0,
                            ap=[[HO * WO, P], [HO * WO * P, 8], [1, HO * WO]],
                        ),
                        in_=bass.AP(
                            tensor=os_.tensor,
                            offset=os_.offset,
                            ap=[po, [HO * WO, 8], [1, HO * WO]],
                        ),
                    )

            # --- tile 10 as two chunks ----------------------------------
            wout10 = wp.tile([P, HO * WO], f32)
            tH10 = tp.tile([P, HO, W], f32)
            obase10 = nfull * HO * WO
            chunk_hw(xa10, tH10, wout10, 0, 6, 0)
            act_scales(wout10, obase10, 0, 6)
            fused_chunk(xc10, wout10, 6, 1, 54)
            act_scales(wout10, obase10, 6, 1)
            # --- tile 11 chunks -----------------------------------------
            wout11 = wp.tile([P, HO * WO], f32)
            tH11 = tp.tile([P, HO, W], f32)
            obase11 = (nfull + 1) * HO * WO
            chunk_hw(xa, tH11, wout11, 0, 3, 0, do_w=False)
            chunk_hw(xb, tH11, wout11, 3, 3, 27, do_w=False, wd=WDB)
            pw11 = list(wout11.ap[0])
            ph11 = list(tH11.ap[0])
            nc.vector.reduce_sum(
                out=bass.AP(
                    tensor=wout11.tensor,
                    offset=wout11.offset,
                    ap=[pw11, [WO, 6], [6, 2]],
                ),
                in_=bass.AP(
                    tensor=tH11.tensor,
                    offset=tH11.offset,
                    ap=[ph11, [W, 6], [41, 2], [1, 7]],
                ),
                axis=X,
            )
            nc.vector.reduce_sum(
                out=bass.AP(
                    tensor=wout11.tensor,
                    offset=wout11.offset + 1,
                    ap=[pw11, [WO, 6], [1, 5]],
                ),
                in_=bass.AP(
                    tensor=tH11.tensor,
                    offset=tH11.offset + 6,
                    ap=[ph11, [W, 6], [7, 5], [1, 8]],
                ),
                axis=X,
            )
            act_scales(wout11, obase11, 0, 6)
            # tiles 8..10 full-block store once tile 10's scales are done
            nc.scalar.dma_start(
                out=bass.AP(
                    tensor=out,
                    offset=8 * HO * WO * P,
                    ap=[[HO * WO, P], [HO * WO * P, 3], [1, HO * WO]],
                ),
                in_=bass.AP(
                    tensor=os_.tensor,
                    offset=os_.offset + 8 * HO * WO,
                    ap=[po, [HO * WO, 3], [1, HO * WO]],
                ),
            )
            # o6: fused reduce + DVE scales + SP store (the tail chain)
            pc = list(xc.ap[0])
            pw = list(wout11.ap[0])
            nc.vector.reduce_sum(
                out=bass.AP(
                    tensor=wout11.tensor,
                    offset=wout11.offset + 42,
                    ap=[pw, [6, 2]],
                ),
                in_=bass.AP(
                    tensor=xc.tensor,
                    offset=xc.offset,
                    ap=[pc, [41, 2], [W, 10], [1, 7]],
                ),
                axis=XY,
            )
            nc.vector.reduce_sum(
                out=bass.AP(
                    tensor=wout11.tensor,
                    offset=wout11.offset + 43,
                    ap=[pw, [1, 5]],
                ),
                in_=bass.AP(
                    tensor=xc.tensor,
                    offset=xc.offset + 6,
                    ap=[pc, [7, 5], [W, 10], [1, 8]],
                ),
                axis=XY,
            )
            nc.vector.tensor_scalar_mul(
                bass.AP(
                    tensor=os_.tensor,
                    offset=os_.offset + obase11 + 42,
                    ap=[po, [6, 2]],
                ),
                bass.AP(
                    tensor=wout11.tensor,
                    offset=wout11.offset + 42,
                    ap=[pw, [6, 2]],
                ),
                1.0 / 70.0,
            )
            nc.vector.tensor_scalar_mul(
                bass.AP(
                    tensor=os_.tensor,
                    offset=os_.offset + obase11 + 43,
                    ap=[po, [1, 5]],
                ),
                bass.AP(
                    tensor=wout11.tensor,
                    offset=wout11.offset + 43,
                    ap=[pw, [1, 5]],
                ),
                1.0 / 80.0,
            )
            nc.sync.dma_start(
                out=out[(nfull + 1) * P :, :],
                in_=os_[:, obase11 : obase11 + HO * WO],
            )
    _strip_pool_ring_memsets(nc)
    _legalize_multiwait(nc)
    _hoist_first_load(nc)
    _strip_final_barrier_round(nc)
    return nc


def kernel(x: np.ndarray) -> np.ndarray:
    global _nc_cache
    from concourse.bass_utils import run_bass_kernel_spmd

    xr = np.ascontiguousarray(
        np.asarray(x, dtype=np.float32).reshape(B * C, H * W)
    )
    if _nc_cache is None:
        _nc_cache = _build()
    nc = _nc_cache
    in_maps = [{"x": xr[k * ROWS : (k + 1) * ROWS]} for k in range(NCORES)]
    res = run_bass_kernel_spmd(nc, in_maps, list(range(NCORES)))
    out = np.concatenate([r["out"] for r in res.results], axis=0)
    return out.reshape(B, C, HO, WO)
